# revision 11
# baseline (speedup 1.0000x reference)
"""Trainium2 Bass kernel for nn_AttentionOutput (complex causal leaky-relu attention).

Reference (B=4, N=4096, F=64), per batch:
    sr = (Qr@Kr^T - Qi@Ki^T)/sqrt(N); si = (Qr@Ki^T + Qi@Kr^T)/sqrt(N)
    wr = tril * leaky_relu(sr);        wi = tril * leaky_relu(si)
    out_r = (wr@Vr)@W_att^T + b;       out_i = (wi@Vi)@W_att^T + b

Distribution: 2 cores per batch.  Core parity h processes j-blocks J === h
(mod 2) for ALL 4096 query rows; causal work is then identical across cores
(slot I needs 2I+2 j-blocks), so a single SPMD program serves all 8 cores and
the host sums the two partial outputs per batch.

Host-side layout prep removes every on-device transpose:
  - scores contract over p = f*2+c (128 partitions, ONE matmul per component):
    sr = Qmodr . K^T where Qmodr = Q with odd columns negated, and
    si = Qmodi . K^T where Qmodi = Q with column pairs swapped; K stays plain.
    Both Q variants are fed pre-transposed [128, N].
  - V' = (1/64) V @ W_att^T folds the score scale and the output projection
    into the attention-value matmul (leaky_relu is positively homogeneous).
  - output is stored transposed ([128, N]: y_r^T on rows 0:64, y_i^T on
    64:128); the host untransposes, interleaves, adds bias, sums parities.

leaky_relu lowering (RELU_CORR): leaky(s) = 0.99*relu(s) + 0.01*s.  For
causally-full j-blocks the 0.01*s term telescopes into a per-slot constant
matmul precomputed on the host (mcr/mci) and accumulated into the y PSUM
bank.  Diagonal tiles compute u = mask*s and w = relu(u), feeding matmuls
against 0.01*V' and 0.99*V'.

v4 perf structure (130us baseline -> target ~70us):
  - y accumulator is ONE [128, 512] PSUM bank: y_r on partitions 0:64
    (PE col-tile T0), y_i on 64:128 (T1).  Value/correction matmuls have 64
    output partitions, so each r/i pair runs CONCURRENTLY on the two column
    halves of the PE array (128x64 col-tiling, tile_position auto-derived
    from out.base_partition()).  The T1 matmul of a pair costs ~4ns.
  - scores for both components live in ONE [128, 1024] two-bank PSUM tile,
    so a full tile needs a single [128,1024] drain instruction (997ns ACT /
    1192ns DVE) instead of two [128,512] ones (686+691ns) — drains then fit
    under the ~650ns/tile PE cadence.  Drains are assigned to ScalarE or
    VectorE by a greedy load-balance over modeled costs.
  - PE instruction order is [s_r(t) | value-pair(t-2) | s_i(t)]: with the
    PE's two weight buffers, every fused LDWEIGHTS then has a full matmul of
    runway and hides completely (215ns/matmul instead of 330).
  - second diagonal j-block per slot only touches columns [256:512) for
    either parity, so its drains and matmuls are narrowed.
  - input DMAs are batched into >=2KB-per-partition-line chunks and ordered
    so slot 0's operands land first.
  - skip_group_check on the y matmuls: the interpreter's zero-region
    bookkeeping mis-handles two col-tile groups (partition ranges 0:64 and
    64:128) in one bank; hardware handles it (validated v2 = baseline
    numerics exactly).

NOTE: ACT Lrelu reading PSUM hangs TRN2 (empirically) — never emit it.
NOTE: PE warmup matmuls into an undrained PSUM bank hang TRN2 — don't.
"""

import numpy as np

import concourse.bacc as bacc
import concourse.tile as tile
from concourse import mybir
from concourse.bass_utils import run_bass_kernel_spmd

B, N, F = 4, 4096, 64
P = 128             # = 2*F: score contraction width / partition count
JB = 128            # j-block width
IBW = 512           # i-block (slot) width
NSLOT = N // IBW    # 8 slots
NJPAR = N // JB // 2  # 16 parity j-blocks per core
NEG = 0.01
SCALE = 1.0 / 64.0  # 1/sqrt(N)
NCORES = 8
LAG = 2             # value matmuls trail scores by LAG tiles (LDW + drain slack)

_DT = mybir.dt.float32
MM_BF16 = True      # bf16 matmul inputs: 4x PE throughput, half the DMA bytes
# modeled engine costs (ns) for the greedy drain balancer
_ACT_FULL = 997     # ACT [128,1024] fp32-PSUM relu drain
_DVE_FULL = 1192    # DVE [128,1024] fp32-PSUM max drain
_CACHE: dict = {}


def _build_nc():
    nc = bacc.Bacc("TRN2", target_bir_lowering=False, num_devices=NCORES)
    dt = _DT
    mdt = mybir.dt.bfloat16 if MM_BF16 else _DT  # matmul input dtype
    qrT = nc.dram_tensor("qrT", [P, N], mdt, kind="ExternalInput")
    qiT = nc.dram_tensor("qiT", [P, N], mdt, kind="ExternalInput")
    kp = nc.dram_tensor("kp", [P, NJPAR * JB], mdt, kind="ExternalInput")
    # va = 0.99 * V' (relu term), vb = 0.01 * V' (raw term, diagonal only)
    var_ = nc.dram_tensor("var", [P, NJPAR * F], mdt, kind="ExternalInput")
    vai = nc.dram_tensor("vai", [P, NJPAR * F], mdt, kind="ExternalInput")
    vbr = nc.dram_tensor("vbr", [P, NJPAR * F], mdt, kind="ExternalInput")
    vbi = nc.dram_tensor("vbi", [P, NJPAR * F], mdt, kind="ExternalInput")
    # per-slot correction weights: 0.01 * sum_{full J} kp_J @ V'_J  [P, 64]
    mcr = nc.dram_tensor("mcr", [P, NSLOT * F], mdt, kind="ExternalInput")
    mci = nc.dram_tensor("mci", [P, NSLOT * F], mdt, kind="ExternalInput")
    dmask = nc.dram_tensor("dmask", [JB, 2 * IBW], mdt, kind="ExternalInput")
    out = nc.dram_tensor("out", [P, N], dt, kind="ExternalOutput")

    relu = mybir.ActivationFunctionType.Relu
    mul_op = mybir.AluOpType.mult

    with tile.TileContext(nc) as tc:
        with (
            tc.tile_pool(name="res", bufs=1) as res,
            tc.tile_pool(name="wp", bufs=4) as wp,
            tc.tile_pool(name="osb", bufs=2) as osb,
            tc.tile_pool(name="spsum", bufs=3, space="PSUM") as spsum,
            tc.tile_pool(name="ypsum", bufs=2, space="PSUM") as ypsum,
        ):
            # --- input DMAs: batched, slot-0-critical data first ---------
            sb_m01 = res.tile([JB, 2 * IBW], mdt, tag="m01")
            sb_qr = res.tile([P, N], mdt, tag="qr")
            sb_qi = res.tile([P, N], mdt, tag="qi")
            sb_k = res.tile([P, NJPAR * JB], mdt, tag="k")
            sb_var = res.tile([P, NJPAR * F], mdt, tag="var")
            sb_vai = res.tile([P, NJPAR * F], mdt, tag="vai")
            sb_vbr = res.tile([P, NJPAR * F], mdt, tag="vbr")
            sb_vbi = res.tile([P, NJPAR * F], mdt, tag="vbi")
            sb_mcr = res.tile([P, NSLOT * F], mdt, tag="mcr")
            sb_mci = res.tile([P, NSLOT * F], mdt, tag="mci")

            nc.sync.dma_start(out=sb_qr[:, 0:512], in_=qrT[:, 0:512])
            nc.sync.dma_start(out=sb_qi[:, 0:512], in_=qiT[:, 0:512])
            nc.scalar.dma_start(out=sb_k[:, 0:1024], in_=kp[:, 0:1024])
            nc.scalar.dma_start(out=sb_m01, in_=dmask[:])
            nc.sync.dma_start(out=sb_qr[:, 512:2048], in_=qrT[:, 512:2048])
            nc.sync.dma_start(out=sb_qi[:, 512:2048], in_=qiT[:, 512:2048])
            nc.scalar.dma_start(out=sb_vbr, in_=vbr[:])
            nc.scalar.dma_start(out=sb_vbi, in_=vbi[:])
            nc.scalar.dma_start(out=sb_var, in_=var_[:])
            nc.scalar.dma_start(out=sb_vai, in_=vai[:])
            nc.sync.dma_start(out=sb_qr[:, 2048:4096], in_=qrT[:, 2048:4096])
            nc.sync.dma_start(out=sb_qi[:, 2048:4096], in_=qiT[:, 2048:4096])
            nc.scalar.dma_start(out=sb_k[:, 1024:2048], in_=kp[:, 1024:2048])
            nc.scalar.dma_start(out=sb_mcr, in_=mcr[:])
            nc.scalar.dma_start(out=sb_mci, in_=mci[:])
            sb_masks = sb_m01

            tasks = [(s, p) for s in range(NSLOT) for p in range(2 * s + 2)]
            state: dict = {}
            load = {"act": 0.0, "dve": 0.0}  # greedy drain balance

            def emit_s_r(s, p):
                isl = slice(s * IBW, (s + 1) * IBW)
                ksl = slice(p * JB, (p + 1) * JB)
                s_pk = spsum.tile([JB, 2 * IBW], dt, tag="s")
                nc.tensor.matmul(s_pk[:, 0:IBW], sb_k[:, ksl], sb_qr[:, isl],
                                 start=True, stop=True)
                state[(s, p)] = s_pk

            def emit_s_i(s, p):
                isl = slice(s * IBW, (s + 1) * IBW)
                ksl = slice(p * JB, (p + 1) * JB)
                s_pk = state[(s, p)]
                nc.tensor.matmul(s_pk[:, IBW:2 * IBW], sb_k[:, ksl],
                                 sb_qi[:, isl], start=True, stop=True)

            def emit_values_a(s, p):
                """Drains + first value matmul pair for tile (s, p)."""
                cnt = 2 * s + 2
                vsl = slice(p * F, (p + 1) * F)
                s_pk = state.pop((s, p))
                if p == 0:
                    y = ypsum.tile([P, IBW], dt, tag="y")
                    state[s] = (y, [False, False])
                y, started = state[s]
                ysub = (y[0:64], y[64:128])
                diag = p >= cnt - 2
                if not diag:
                    # one merged relu drain for both components
                    w = wp.tile([JB, 2 * IBW], mdt, tag="w")
                    if load["act"] + _ACT_FULL <= load["dve"] + _DVE_FULL:
                        load["act"] += _ACT_FULL
                        nc.scalar.activation(w[:], s_pk[:], relu)
                    else:
                        load["dve"] += _DVE_FULL
                        nc.vector.tensor_scalar_max(w[:], s_pk[:], 0.0)
                    for ci in range(2):
                        first = not started[ci]
                        started[ci] = True
                        sb_va = sb_var if ci == 0 else sb_vai
                        nc.tensor.matmul(ysub[ci][:], sb_va[:, vsl],
                                         w[:, ci * IBW:(ci + 1) * IBW],
                                         start=first, stop=False,
                                         skip_group_check=True)
                else:
                    # diagonal: u = mask*s (merged across components for the
                    # first diag block; the second only affects i >= 256)
                    k_idx = p - (cnt - 2)
                    lo = 0 if (k_idx == 0 or not started[0]) else IBW // 2
                    msl = slice(k_idx * IBW + lo, (k_idx + 1) * IBW)
                    u = wp.tile([JB, 2 * IBW], mdt, tag="u")
                    for ci in range(2):
                        csl = slice(ci * IBW + lo, (ci + 1) * IBW)
                        load["dve"] += (120 + IBW - lo) / 0.96
                        nc.vector.tensor_tensor(out=u[:, csl],
                                                in0=s_pk[:, csl],
                                                in1=sb_masks[:, msl], op=mul_op)
                    state[(s, p, "u")] = (u, lo)
                    for ci in range(2):
                        first = not started[ci]
                        started[ci] = True
                        sb_vb = sb_vbr if ci == 0 else sb_vbi
                        csl = slice(lo, IBW)
                        nc.tensor.matmul(ysub[ci][:, csl], sb_vb[:, vsl],
                                         u[:, ci * IBW + lo:(ci + 1) * IBW],
                                         start=first, stop=False,
                                         skip_group_check=True)

            def emit_values_b(s, p):
                """Second (diag) pair + correction + evacuation for (s, p)."""
                cnt = 2 * s + 2
                isl = slice(s * IBW, (s + 1) * IBW)
                vsl = slice(p * F, (p + 1) * F)
                y, started = state[s]
                ysub = (y[0:64], y[64:128])
                diag = p >= cnt - 2
                if diag:
                    u, lo = state.pop((s, p, "u"))
                    w = wp.tile([JB, 2 * IBW], mdt, tag="w")
                    if lo == 0:
                        load["dve"] += (58 + 1024 / 4) / 0.96
                        nc.vector.tensor_scalar_max(w[:], u[:], 0.0)
                    else:
                        for ci in range(2):
                            csl = slice(ci * IBW + lo, (ci + 1) * IBW)
                            load["act"] += (224 + IBW - lo) / 1.2
                            nc.scalar.activation(w[:, csl], u[:, csl], relu)
                    last = (s == 0 and p == cnt - 1)
                    for ci in range(2):
                        sb_va = sb_var if ci == 0 else sb_vai
                        csl = slice(lo, IBW)
                        nc.tensor.matmul(ysub[ci][:, csl], sb_va[:, vsl],
                                         w[:, ci * IBW + lo:(ci + 1) * IBW],
                                         start=False, stop=last,
                                         skip_group_check=True)
                if p == cnt - 1:
                    if s > 0:
                        msl = slice(s * F, (s + 1) * F)
                        nc.tensor.matmul(y[0:64], sb_mcr[:, msl],
                                         sb_qr[:, isl], start=False, stop=True,
                                         skip_group_check=True)
                        nc.tensor.matmul(y[64:128], sb_mci[:, msl],
                                         sb_qi[:, isl], start=False, stop=True,
                                         skip_group_check=True)
                    y_sb = osb.tile([P, IBW], dt, tag="ysb")
                    load["act"] += (172 + 256) / 1.2
                    load["dve"] += (120 + 256) / 0.96
                    nc.scalar.copy(y_sb[:, 0:256], y[:, 0:256])
                    nc.vector.tensor_copy(y_sb[:, 256:512], y[:, 256:512])
                    nc.sync.dma_start(out=out[:, isl], in_=y_sb[:])
                    del state[s]

            # pipeline in 2-tile groups: [scores(t) scores(t+1) |
            # values(t-2) values(t-1)].  Scores are 128-col-mode matmuls,
            # value pairs 64-col-mode; batching halves the PE column-mode
            # switches (~110ns each).  Score PSUM packs are freed by their
            # drains, so only ~3 packs are ever live.
            assert len(tasks) % 2 == 0
            for t in range(0, len(tasks), 2):
                emit_s_r(*tasks[t])
                emit_s_i(*tasks[t])
                emit_s_r(*tasks[t + 1])
                emit_s_i(*tasks[t + 1])
                if t >= 2:
                    emit_values_a(*tasks[t - 2])
                    emit_values_b(*tasks[t - 2])
                    emit_values_a(*tasks[t - 1])
                    emit_values_b(*tasks[t - 1])
            for t in range(len(tasks) - 2, len(tasks)):
                emit_values_a(*tasks[t])
                emit_values_b(*tasks[t])
    nc.compile()
    return nc


def _prep_inputs(Q, K, V, W_att, b_att):
    """Host-side re-layout: per-core in_maps for run_bass_kernel_spmd."""
    Q = np.asarray(Q, dtype=np.float32)
    K = np.asarray(K, dtype=np.float32)
    V = np.asarray(V, dtype=np.float32)
    W_att = np.asarray(W_att, dtype=np.float32)

    Qf = Q.reshape(B, N, P)          # [b, i, f*2+c]
    Kf = K.reshape(B, N, P)
    Vpr = SCALE * (V[..., 0] @ W_att.T)   # [B, N, F]
    Vpi = SCALE * (V[..., 1] @ W_att.T)

    # causal masks for a slot's last two parity j-blocks, per core parity h:
    # diagonal sub-block d = 2k+h of the slot's group of 4
    jj = np.arange(JB)[:, None]
    ii = np.arange(IBW)[None, :]
    masks = {h: np.concatenate(
        [(ii >= jj + JB * (2 * k + h)).astype(np.float32) for k in range(2)],
        axis=1) for h in (0, 1)}

    if MM_BF16:
        import ml_dtypes
        cvt = lambda a: np.ascontiguousarray(a).astype(ml_dtypes.bfloat16)
    else:
        cvt = lambda a: np.ascontiguousarray(a, dtype=np.float32)

    in_maps = []
    for c in range(NCORES):
        b, h = divmod(c, 2)
        Qmodr = Qf[b].copy()
        Qmodr[:, 1::2] *= -1.0
        Qmodi = np.empty_like(Qf[b])
        Qmodi[:, 0::2] = Qf[b][:, 1::2]
        Qmodi[:, 1::2] = Qf[b][:, 0::2]
        # parity-packed K: [P, NJPAR*JB], position pp holds block J = 2*pp+h
        kp3 = Kf[b].reshape(N // JB, JB, P)[h::2]          # [16, j, p]
        kp = kp3.transpose(2, 0, 1).reshape(P, -1)         # [p, pp*JB+j]
        vr3 = Vpr[b].reshape(N // JB, JB, F)[h::2]         # [16, j, f]
        vi3 = Vpi[b].reshape(N // JB, JB, F)[h::2]
        vpr = vr3.transpose(1, 0, 2).reshape(JB, -1)       # [j, pp*F+f]
        vpi = vi3.transpose(1, 0, 2).reshape(JB, -1)
        # per-slot correction: 0.01 * sum over FULL blocks (pos < cnt-2 = 2s)
        prod_r = np.einsum('bjp,bjf->bpf', kp3, vr3)       # [16, p, f]
        prod_i = np.einsum('bjp,bjf->bpf', kp3, vi3)
        pre_r = np.concatenate(
            [np.zeros((1, P, F), np.float32), np.cumsum(prod_r, axis=0)])
        pre_i = np.concatenate(
            [np.zeros((1, P, F), np.float32), np.cumsum(prod_i, axis=0)])
        mcr = np.concatenate([NEG * pre_r[2 * s] for s in range(NSLOT)], axis=1)
        mci = np.concatenate([NEG * pre_i[2 * s] for s in range(NSLOT)], axis=1)
        in_maps.append({
            "qrT": cvt(Qmodr.T),
            "qiT": cvt(Qmodi.T),
            "kp": cvt(kp),
            "var": cvt((1.0 - NEG) * vpr),
            "vai": cvt((1.0 - NEG) * vpi),
            "vbr": cvt(NEG * vpr),
            "vbi": cvt(NEG * vpi),
            "mcr": cvt(mcr),
            "mci": cvt(mci),
            "dmask": cvt(masks[h]),
        })
    return in_maps


def _gather(results, b_att):
    b_att = np.asarray(b_att, dtype=np.float32)
    out = np.empty((B, N, F, 2), dtype=np.float32)
    for b in range(B):
        y = results[2 * b]["out"] + results[2 * b + 1]["out"]  # [128, N]
        out[b, :, :, 0] = y[0:64].T + b_att[None, :]
        out[b, :, :, 1] = y[64:128].T + b_att[None, :]
    return out


def kernel(Q, K, V, W_att, b_att):
    if "nc" not in _CACHE:
        _CACHE["nc"] = _build_nc()
    nc = _CACHE["nc"]
    in_maps = _prep_inputs(Q, K, V, W_att, b_att)
    res = run_bass_kernel_spmd(nc, in_maps, core_ids=list(range(NCORES)))
    return _gather(res.results, b_att)


# revision 13
# speedup vs baseline: 1.2093x; 1.2093x over previous
"""Trainium2 Bass kernel for nn_AttentionOutput (complex causal leaky-relu attention).

Reference (B=4, N=4096, F=64), per batch:
    sr = (Qr@Kr^T - Qi@Ki^T)/sqrt(N); si = (Qr@Ki^T + Qi@Kr^T)/sqrt(N)
    wr = tril * leaky_relu(sr);        wi = tril * leaky_relu(si)
    out_r = (wr@Vr)@W_att^T + b;       out_i = (wi@Vi)@W_att^T + b

Distribution: 2 cores per batch.  Core parity h processes j-blocks J === h
(mod 2) for ALL 4096 query rows; causal work is then identical across cores
(slot I needs 2I+2 j-blocks), so a single SPMD program serves all 8 cores and
the host sums the two partial outputs per batch.

Host-side layout prep removes every on-device transpose:
  - scores contract over p = f*2+c (128 partitions, ONE matmul per component):
    sr = Qmodr . K^T where Qmodr = Q with odd columns negated, and
    si = Qmodi . K^T where Qmodi = Q with column pairs swapped; K stays plain.
    Both Q variants are fed pre-transposed [128, N].
  - V' = (1/64) V @ W_att^T folds the score scale and the output projection
    into the attention-value matmul (leaky_relu is positively homogeneous).
  - output is stored transposed ([128, N]: y_r^T on rows 0:64, y_i^T on
    64:128); the host untransposes, interleaves, adds bias, sums parities.

leaky_relu lowering (RELU_CORR): leaky(s) = 0.99*relu(s) + 0.01*s.  For
causally-full j-blocks the 0.01*s term telescopes into a per-slot constant
matmul precomputed on the host (mcr/mci) and accumulated into the y PSUM
bank.  Diagonal tiles compute u = mask*s and w = relu(u), feeding matmuls
against 0.01*V' and 0.99*V'.

v8 perf structure (130us baseline -> 90us measured at full clock; device
sometimes sits in a P0 downclock state where the PE runs 2.0 instead of
2.4 GHz and the same kernel measures ~105us):
  - y accumulator is ONE [128, 512] PSUM bank: y_r on partitions 0:64
    (PE col-tile T0), y_i on 64:128 (T1).  Value/correction matmuls have 64
    output partitions, so each r/i pair runs CONCURRENTLY on the two column
    halves of the PE array (128x64 col-tiling, tile_position auto-derived
    from out.base_partition()).  The T1 matmul of a pair costs ~4ns.
  - scores for both components live in ONE [128, 1024] two-bank PSUM tile,
    so a full tile needs a single [128,1024] drain instruction (997ns ACT /
    1192ns DVE) instead of two [128,512] ones (686+691ns) — drains then fit
    under the ~650ns/tile PE cadence.  Drains are assigned to ScalarE or
    VectorE by a greedy load-balance over modeled costs.
  - PE instructions are emitted in 2-tile groups [scores(t) scores(t+1) |
    values(t-2) values(t-1)]: scores are 128-col-mode matmuls, value pairs
    64-col-mode, and each col-mode switch stalls the PE ~110ns, so batching
    halves the switch count.  (A [s_r | pair | s_i] interleave is WORSE: the
    pair's weights evict kp from the PE's two weight buffers.)
  - second diagonal j-block per slot only touches columns [256:512) for
    either parity, so its drains and matmuls are narrowed.
  - input DMAs are batched into >=2KB-per-partition-line chunks and ordered
    so slot 0's operands land first.
  - skip_group_check on the y matmuls: the interpreter's zero-region
    bookkeeping mis-handles two col-tile groups (partition ranges 0:64 and
    64:128) in one bank; hardware handles it (validated v2 = baseline
    numerics exactly).

NOTE: ACT Lrelu reading PSUM hangs TRN2 (empirically) — never emit it.
NOTE: PE warmup matmuls into an undrained PSUM bank hang TRN2 — don't.
"""

import numpy as np

import concourse.bacc as bacc
import concourse.tile as tile
from concourse import mybir
from concourse.bass_utils import run_bass_kernel_spmd

B, N, F = 4, 4096, 64
P = 128             # = 2*F: score contraction width / partition count
JB = 128            # j-block width
IBW = 512           # i-block (slot) width
NSLOT = N // IBW    # 8 slots
NJPAR = N // JB // 2  # 16 parity j-blocks per core
NEG = 0.01
SCALE = 1.0 / 64.0  # 1/sqrt(N)
NCORES = 8
LAG = 2             # value matmuls trail scores by LAG tiles (LDW + drain slack)

_DT = mybir.dt.float32
MM_BF16 = True      # bf16 matmul inputs: 4x PE throughput, half the DMA bytes
# modeled engine costs (ns) for the greedy drain balancer
_ACT_FULL = 997     # ACT [128,1024] fp32-PSUM relu drain
_DVE_FULL = 1192    # DVE [128,1024] fp32-PSUM max drain
_CACHE: dict = {}


def _build_nc():
    nc = bacc.Bacc("TRN2", target_bir_lowering=False, num_devices=NCORES)
    dt = _DT
    mdt = mybir.dt.bfloat16 if MM_BF16 else _DT  # matmul input dtype
    qrT = nc.dram_tensor("qrT", [P, N], mdt, kind="ExternalInput")
    qiT = nc.dram_tensor("qiT", [P, N], mdt, kind="ExternalInput")
    kp = nc.dram_tensor("kp", [P, NJPAR * JB], mdt, kind="ExternalInput")
    # va = 0.99 * V' (relu term), vb = 0.01 * V' (raw term, diagonal only)
    var_ = nc.dram_tensor("var", [P, NJPAR * F], mdt, kind="ExternalInput")
    vai = nc.dram_tensor("vai", [P, NJPAR * F], mdt, kind="ExternalInput")
    vbr = nc.dram_tensor("vbr", [P, NJPAR * F], mdt, kind="ExternalInput")
    vbi = nc.dram_tensor("vbi", [P, NJPAR * F], mdt, kind="ExternalInput")
    # per-slot correction weights: 0.01 * sum_{full J} kp_J @ V'_J  [P, 64]
    mcr = nc.dram_tensor("mcr", [P, NSLOT * F], mdt, kind="ExternalInput")
    mci = nc.dram_tensor("mci", [P, NSLOT * F], mdt, kind="ExternalInput")
    dmask = nc.dram_tensor("dmask", [JB, 2 * IBW], mdt, kind="ExternalInput")
    out = nc.dram_tensor("out", [P, N], dt, kind="ExternalOutput")

    relu = mybir.ActivationFunctionType.Relu
    mul_op = mybir.AluOpType.mult

    with tile.TileContext(nc) as tc:
        with (
            tc.tile_pool(name="res", bufs=1) as res,
            tc.tile_pool(name="wp", bufs=4) as wp,
            tc.tile_pool(name="osb", bufs=2) as osb,
            tc.tile_pool(name="spsum", bufs=3, space="PSUM") as spsum,
            tc.tile_pool(name="ypsum", bufs=2, space="PSUM") as ypsum,
        ):
            # --- input DMAs: batched, slot-0-critical data first ---------
            sb_m01 = res.tile([JB, 2 * IBW], mdt, tag="m01")
            sb_qr = res.tile([P, N], mdt, tag="qr")
            sb_qi = res.tile([P, N], mdt, tag="qi")
            sb_k = res.tile([P, NJPAR * JB], mdt, tag="k")
            sb_var = res.tile([P, NJPAR * F], mdt, tag="var")
            sb_vai = res.tile([P, NJPAR * F], mdt, tag="vai")
            sb_vbr = res.tile([P, NJPAR * F], mdt, tag="vbr")
            sb_vbi = res.tile([P, NJPAR * F], mdt, tag="vbi")
            sb_mcr = res.tile([P, NSLOT * F], mdt, tag="mcr")
            sb_mci = res.tile([P, NSLOT * F], mdt, tag="mci")

            nc.sync.dma_start(out=sb_qr[:, 0:512], in_=qrT[:, 0:512])
            nc.sync.dma_start(out=sb_qi[:, 0:512], in_=qiT[:, 0:512])
            nc.scalar.dma_start(out=sb_k[:, 0:512], in_=kp[:, 0:512])
            nc.scalar.dma_start(out=sb_m01, in_=dmask[:])
            nc.sync.dma_start(out=sb_qr[:, 512:2048], in_=qrT[:, 512:2048])
            nc.sync.dma_start(out=sb_qi[:, 512:2048], in_=qiT[:, 512:2048])
            nc.scalar.dma_start(out=sb_vbr, in_=vbr[:])
            nc.scalar.dma_start(out=sb_vbi, in_=vbi[:])
            nc.scalar.dma_start(out=sb_var, in_=var_[:])
            nc.scalar.dma_start(out=sb_vai, in_=vai[:])
            nc.sync.dma_start(out=sb_qr[:, 2048:4096], in_=qrT[:, 2048:4096])
            nc.sync.dma_start(out=sb_qi[:, 2048:4096], in_=qiT[:, 2048:4096])
            nc.scalar.dma_start(out=sb_k[:, 512:2048], in_=kp[:, 512:2048])
            nc.scalar.dma_start(out=sb_mcr, in_=mcr[:])
            nc.scalar.dma_start(out=sb_mci, in_=mci[:])
            sb_masks = sb_m01

            tasks = [(s, p) for s in range(NSLOT) for p in range(2 * s + 2)]
            state: dict = {}
            load = {"act": 0.0, "dve": 0.0}  # greedy drain balance

            def emit_s_r(s, p):
                isl = slice(s * IBW, (s + 1) * IBW)
                ksl = slice(p * JB, (p + 1) * JB)
                s_pk = spsum.tile([JB, 2 * IBW], dt, tag="s")
                nc.tensor.matmul(s_pk[:, 0:IBW], sb_k[:, ksl], sb_qr[:, isl],
                                 start=True, stop=True)
                state[(s, p)] = s_pk

            def emit_s_i(s, p):
                isl = slice(s * IBW, (s + 1) * IBW)
                ksl = slice(p * JB, (p + 1) * JB)
                s_pk = state[(s, p)]
                nc.tensor.matmul(s_pk[:, IBW:2 * IBW], sb_k[:, ksl],
                                 sb_qi[:, isl], start=True, stop=True)

            def emit_values_a(s, p):
                """Drains + first value matmul pair for tile (s, p)."""
                cnt = 2 * s + 2
                vsl = slice(p * F, (p + 1) * F)
                s_pk = state.pop((s, p))
                if p == 0:
                    y = ypsum.tile([P, IBW], dt, tag="y")
                    state[s] = (y, [False, False])
                y, started = state[s]
                ysub = (y[0:64], y[64:128])
                diag = p >= cnt - 2
                if not diag:
                    # one merged relu drain for both components
                    w = wp.tile([JB, 2 * IBW], mdt, tag="w")
                    if load["act"] + _ACT_FULL <= load["dve"] + _DVE_FULL:
                        load["act"] += _ACT_FULL
                        nc.scalar.activation(w[:], s_pk[:], relu)
                    else:
                        load["dve"] += _DVE_FULL
                        nc.vector.tensor_scalar_max(w[:], s_pk[:], 0.0)
                    for ci in range(2):
                        first = not started[ci]
                        started[ci] = True
                        sb_va = sb_var if ci == 0 else sb_vai
                        nc.tensor.matmul(ysub[ci][:], sb_va[:, vsl],
                                         w[:, ci * IBW:(ci + 1) * IBW],
                                         start=first, stop=False,
                                         skip_group_check=True)
                else:
                    # diagonal: u = mask*s (merged across components for the
                    # first diag block; the second only affects i >= 256)
                    k_idx = p - (cnt - 2)
                    lo = 0 if (k_idx == 0 or not started[0]) else IBW // 2
                    msl = slice(k_idx * IBW + lo, (k_idx + 1) * IBW)
                    u = wp.tile([JB, 2 * IBW], mdt, tag="u")
                    for ci in range(2):
                        csl = slice(ci * IBW + lo, (ci + 1) * IBW)
                        load["dve"] += (120 + IBW - lo) / 0.96
                        nc.vector.tensor_tensor(out=u[:, csl],
                                                in0=s_pk[:, csl],
                                                in1=sb_masks[:, msl], op=mul_op)
                    state[(s, p, "u")] = (u, lo)
                    for ci in range(2):
                        first = not started[ci]
                        started[ci] = True
                        sb_vb = sb_vbr if ci == 0 else sb_vbi
                        csl = slice(lo, IBW)
                        nc.tensor.matmul(ysub[ci][:, csl], sb_vb[:, vsl],
                                         u[:, ci * IBW + lo:(ci + 1) * IBW],
                                         start=first, stop=False,
                                         skip_group_check=True)

            def emit_values_b(s, p):
                """Second (diag) pair + correction + evacuation for (s, p)."""
                cnt = 2 * s + 2
                isl = slice(s * IBW, (s + 1) * IBW)
                vsl = slice(p * F, (p + 1) * F)
                y, started = state[s]
                ysub = (y[0:64], y[64:128])
                diag = p >= cnt - 2
                if diag:
                    u, lo = state.pop((s, p, "u"))
                    w = wp.tile([JB, 2 * IBW], mdt, tag="w")
                    if lo == 0:
                        load["dve"] += (58 + 1024 / 4) / 0.96
                        nc.vector.tensor_scalar_max(w[:], u[:], 0.0)
                    else:
                        for ci in range(2):
                            csl = slice(ci * IBW + lo, (ci + 1) * IBW)
                            load["act"] += (224 + IBW - lo) / 1.2
                            nc.scalar.activation(w[:, csl], u[:, csl], relu)
                    last = (s == 0 and p == cnt - 1)
                    for ci in range(2):
                        sb_va = sb_var if ci == 0 else sb_vai
                        csl = slice(lo, IBW)
                        nc.tensor.matmul(ysub[ci][:, csl], sb_va[:, vsl],
                                         w[:, ci * IBW + lo:(ci + 1) * IBW],
                                         start=False, stop=last,
                                         skip_group_check=True)
                if p == cnt - 1:
                    if s > 0:
                        msl = slice(s * F, (s + 1) * F)
                        nc.tensor.matmul(y[0:64], sb_mcr[:, msl],
                                         sb_qr[:, isl], start=False, stop=True,
                                         skip_group_check=True)
                        nc.tensor.matmul(y[64:128], sb_mci[:, msl],
                                         sb_qi[:, isl], start=False, stop=True,
                                         skip_group_check=True)
                    y_sb = osb.tile([P, IBW], dt, tag="ysb")
                    load["act"] += (172 + 256) / 1.2
                    load["dve"] += (120 + 256) / 0.96
                    h0 = slice(s * IBW, s * IBW + 256)
                    h1 = slice(s * IBW + 256, (s + 1) * IBW)
                    nc.scalar.copy(y_sb[:, 0:256], y[:, 0:256])
                    nc.sync.dma_start(out=out[:, h0], in_=y_sb[:, 0:256])
                    nc.vector.tensor_copy(y_sb[:, 256:512], y[:, 256:512])
                    nc.sync.dma_start(out=out[:, h1], in_=y_sb[:, 256:512])
                    del state[s]

            # pipeline in 2-tile groups: [scores(t) scores(t+1) |
            # values(t-2) values(t-1)].  Scores are 128-col-mode matmuls,
            # value pairs 64-col-mode; batching halves the PE column-mode
            # switches (~110ns each).  Score PSUM packs are freed by their
            # drains, so only ~3 packs are ever live.
            G = 3
            assert len(tasks) % G == 0
            for t in range(0, len(tasks), G):
                for j in range(G):
                    emit_s_r(*tasks[t + j])
                    emit_s_i(*tasks[t + j])
                if t >= G:
                    for j in range(G):
                        emit_values_a(*tasks[t - G + j])
                        emit_values_b(*tasks[t - G + j])
            for t in range(len(tasks) - G, len(tasks)):
                emit_values_a(*tasks[t])
                emit_values_b(*tasks[t])
    nc.compile()
    return nc


def _prep_inputs(Q, K, V, W_att, b_att):
    """Host-side re-layout: per-core in_maps for run_bass_kernel_spmd."""
    Q = np.asarray(Q, dtype=np.float32)
    K = np.asarray(K, dtype=np.float32)
    V = np.asarray(V, dtype=np.float32)
    W_att = np.asarray(W_att, dtype=np.float32)

    Qf = Q.reshape(B, N, P)          # [b, i, f*2+c]
    Kf = K.reshape(B, N, P)
    Vpr = SCALE * (V[..., 0] @ W_att.T)   # [B, N, F]
    Vpi = SCALE * (V[..., 1] @ W_att.T)

    # causal masks for a slot's last two parity j-blocks, per core parity h:
    # diagonal sub-block d = 2k+h of the slot's group of 4
    jj = np.arange(JB)[:, None]
    ii = np.arange(IBW)[None, :]
    masks = {h: np.concatenate(
        [(ii >= jj + JB * (2 * k + h)).astype(np.float32) for k in range(2)],
        axis=1) for h in (0, 1)}

    if MM_BF16:
        import ml_dtypes
        cvt = lambda a: np.ascontiguousarray(a).astype(ml_dtypes.bfloat16)
    else:
        cvt = lambda a: np.ascontiguousarray(a, dtype=np.float32)

    in_maps = []
    for c in range(NCORES):
        b, h = divmod(c, 2)
        Qmodr = Qf[b].copy()
        Qmodr[:, 1::2] *= -1.0
        Qmodi = np.empty_like(Qf[b])
        Qmodi[:, 0::2] = Qf[b][:, 1::2]
        Qmodi[:, 1::2] = Qf[b][:, 0::2]
        # parity-packed K: [P, NJPAR*JB], position pp holds block J = 2*pp+h
        kp3 = Kf[b].reshape(N // JB, JB, P)[h::2]          # [16, j, p]
        kp = kp3.transpose(2, 0, 1).reshape(P, -1)         # [p, pp*JB+j]
        vr3 = Vpr[b].reshape(N // JB, JB, F)[h::2]         # [16, j, f]
        vi3 = Vpi[b].reshape(N // JB, JB, F)[h::2]
        vpr = vr3.transpose(1, 0, 2).reshape(JB, -1)       # [j, pp*F+f]
        vpi = vi3.transpose(1, 0, 2).reshape(JB, -1)
        # per-slot correction: 0.01 * sum over FULL blocks (pos < cnt-2 = 2s)
        prod_r = np.einsum('bjp,bjf->bpf', kp3, vr3)       # [16, p, f]
        prod_i = np.einsum('bjp,bjf->bpf', kp3, vi3)
        pre_r = np.concatenate(
            [np.zeros((1, P, F), np.float32), np.cumsum(prod_r, axis=0)])
        pre_i = np.concatenate(
            [np.zeros((1, P, F), np.float32), np.cumsum(prod_i, axis=0)])
        mcr = np.concatenate([NEG * pre_r[2 * s] for s in range(NSLOT)], axis=1)
        mci = np.concatenate([NEG * pre_i[2 * s] for s in range(NSLOT)], axis=1)
        in_maps.append({
            "qrT": cvt(Qmodr.T),
            "qiT": cvt(Qmodi.T),
            "kp": cvt(kp),
            "var": cvt((1.0 - NEG) * vpr),
            "vai": cvt((1.0 - NEG) * vpi),
            "vbr": cvt(NEG * vpr),
            "vbi": cvt(NEG * vpi),
            "mcr": cvt(mcr),
            "mci": cvt(mci),
            "dmask": cvt(masks[h]),
        })
    return in_maps


def _gather(results, b_att):
    b_att = np.asarray(b_att, dtype=np.float32)
    out = np.empty((B, N, F, 2), dtype=np.float32)
    for b in range(B):
        y = results[2 * b]["out"] + results[2 * b + 1]["out"]  # [128, N]
        out[b, :, :, 0] = y[0:64].T + b_att[None, :]
        out[b, :, :, 1] = y[64:128].T + b_att[None, :]
    return out


def kernel(Q, K, V, W_att, b_att):
    if "nc" not in _CACHE:
        _CACHE["nc"] = _build_nc()
    nc = _CACHE["nc"]
    in_maps = _prep_inputs(Q, K, V, W_att, b_att)
    res = run_bass_kernel_spmd(nc, in_maps, core_ids=list(range(NCORES)))
    return _gather(res.results, b_att)
